# revision 1
# baseline (speedup 1.0000x reference)
"""Trainium2 Bass kernel for nn_MultiHeadAttention (RoPE MHA, B=2 S=2048 E=1024 H=16).

Sharding: tensor-parallel over heads — 2 heads per core on 8 cores. Each core
computes its heads' q/k/v projections, RoPE, attention, and the partial output
projection (its rows of Wo); the host sums the 8 partials and adds bo.

Device layouts: q/k as [d, token] (transposed) so attention scores come out as
[ks, qs]; softmax's row-sum falls out of the same matmul that computes ctx via
a ones column appended to v. Normalization multiplies by 1/Z broadcast across
partitions via a DRAM bounce. rotate_half is a signed-permutation matmul.
v is projected transposed (full-width matmuls) and PE-transposed to natural.

Engine balance: exp on ACT (the stage-2 pacer), projection evictions on ACT,
RoPE products + normalize on DVE, RoPE final add on GpSimd, DMAs spread over
sync/scalar/vector/gpsimd queues. Two cross-stage PSUM pools (no stage
barriers): A = 2x2-bank slots (psq/psk/pss), B = 4x1-bank slots (rest).
"""

import os
import sys
from contextlib import ExitStack

import numpy as np

for _p in ("/opt/trn_rl_repo", "/opt/pypackages"):
    if _p not in sys.path and os.path.isdir(_p):
        sys.path.append(_p)

import concourse.bass as bass
import concourse.mybir as mybir
import concourse.tile as tile
from concourse import bacc
from concourse import bass_utils
from concourse.masks import make_identity

F32 = mybir.dt.float32
AF = mybir.ActivationFunctionType
OP = mybir.AluOpType

B = 2
S = 2048
E = 1024
H = 16
D = 64
N_CORES = 8
HPC = H // N_CORES  # heads per core = 2
HD = HPC * D  # 128

MM_MODE = os.environ.get("MHA_MM_MODE", "bf16")  # 'f32' | 'f32r' | 'bf16'

LAST_RESULTS = None  # BassKernelResults of the most recent run (for test harness)
_NC_CACHE = {}


def build_mha_nc(mm_mode):
    T = B * S
    TC = 512  # token chunk for projections
    NCH = T // TC
    QC = min(512, S)  # query chunk in attention (N<=512: one PSUM bank per matmul)
    NQC = S // QC
    NKT = S // 128  # key tiles per batch
    KE = E // 128  # contraction tiles for projections

    dt_in = {"bf16": mybir.dt.bfloat16, "f32r": mybir.dt.float32r, "f32": F32}[mm_mode]

    nc = bacc.Bacc(None, target_bir_lowering=False, debug=False)

    xT = nc.dram_tensor("xT", [E, T], dt_in, kind="ExternalInput")
    wq = nc.dram_tensor("wq", [E, HD], dt_in, kind="ExternalInput")
    wk = nc.dram_tensor("wk", [E, HD], dt_in, kind="ExternalInput")
    wv = nc.dram_tensor("wv", [E, HD], dt_in, kind="ExternalInput")
    bq = nc.dram_tensor("bq", [HD, 1], F32, kind="ExternalInput")
    bk = nc.dram_tensor("bk", [HD, 1], F32, kind="ExternalInput")
    bv = nc.dram_tensor("bv", [HD, 1], F32, kind="ExternalInput")
    wo = nc.dram_tensor("wo", [HD, E], dt_in, kind="ExternalInput")
    cosT = nc.dram_tensor("cosT", [HD, T], F32, kind="ExternalInput")
    sinT = nc.dram_tensor("sinT", [HD, T], F32, kind="ExternalInput")
    rot = nc.dram_tensor("rot", [HD, HD], dt_in, kind="ExternalInput")
    ones = nc.dram_tensor("ones", [1, 1], dt_in, kind="ExternalInput")
    yp = nc.dram_tensor("yp", [T, E], F32, kind="ExternalOutput")

    scale = 1.0 / np.sqrt(D)

    with tile.TileContext(nc) as tc, ExitStack() as ctx:
        const = ctx.enter_context(tc.tile_pool(name="const", bufs=1))
        xt_pool = ctx.enter_context(tc.tile_pool(name="xt", bufs=2 * KE))
        cs_pool = ctx.enter_context(tc.tile_pool(name="cs", bufs=4))
        qkraw_pool = ctx.enter_context(tc.tile_pool(name="qkraw", bufs=4))
        rope_tmp = ctx.enter_context(tc.tile_pool(name="ropetmp", bufs=4))
        persist = ctx.enter_context(tc.tile_pool(name="persist", bufs=1))
        exps_pool = ctx.enter_context(tc.tile_pool(name="exps", bufs=8))
        zr_pool = ctx.enter_context(tc.tile_pool(name="zr", bufs=6))
        zb_pool = ctx.enter_context(tc.tile_pool(name="zb", bufs=6))
        osb_pool = ctx.enter_context(tc.tile_pool(name="osb", bufs=6))
        csh_pool = ctx.enter_context(tc.tile_pool(name="csh", bufs=4))
        dram = ctx.enter_context(tc.tile_pool(name="dram", bufs=8, space="DRAM"))

        # cross-stage PSUM pools: A = 2 slots x 2 banks, B = 4 slots x 1 bank
        ps_a = ctx.enter_context(tc.tile_pool(name="ps_a", bufs=2, space="PSUM"))
        ps_b = ctx.enter_context(tc.tile_pool(name="ps_b", bufs=4, space="PSUM"))
        ps_c = ps_b

        # ---- constants to SBUF (gpsimd queue; off the sync DMA path) ----
        def load_const(name, dram_t, shape, dt):
            t = const.tile(shape, dt, name=name, tag=name)
            nc.gpsimd.dma_start(t[:], dram_t.ap())
            return t

        wq_sb = [None] * KE
        wk_sb = [None] * KE
        wv_sb = [None] * KE
        for k in range(KE):
            for nm, dr, arr in (("wq", wq, wq_sb), ("wk", wk, wk_sb), ("wv", wv, wv_sb)):
                t = const.tile([128, HD], dt_in, name=f"{nm}_{k}", tag=f"{nm}_{k}")
                nc.gpsimd.dma_start(t[:], dr.ap()[128 * k : 128 * (k + 1), :])
                arr[k] = t
        bq_sb = load_const("bq_sb", bq, [HD, 1], F32)
        bk_sb = load_const("bk_sb", bk, [HD, 1], F32)
        bv_sb = load_const("bv_sb", bv, [HD, 1], F32)
        wo_sb = load_const("wo_sb", wo, [HD, E], dt_in)
        rot_sb = load_const("rot_sb", rot, [HD, HD], dt_in)
        ident = const.tile([128, 128], dt_in, name="ident", tag="ident")
        make_identity(nc, ident)

        # ---- persistent intermediates ----
        q_rope = persist.tile([HD, T], dt_in, name="q_rope", tag="q_rope")
        k_rope = persist.tile([HD, T], dt_in, name="k_rope", tag="k_rope")
        v_sb = []
        ones_ap = ones.ap()
        for i in range(T // 128):
            t = persist.tile([128, HPC * (D + 1)], dt_in, name=f"v_{i}", tag=f"v_{i}")
            for h in range(HPC):
                sl_ones = t[:, (D + 1) * h + D : (D + 1) * (h + 1)]
                if mm_mode == "f32r":
                    nc.gpsimd.dma_start(
                        sl_ones,
                        bass.AP(tensor=ones_ap.tensor, offset=ones_ap.offset, ap=[[0, 128], [1, 1]]),
                    )
                else:
                    nc.vector.memset(sl_ones, 1.0)
            v_sb.append(t)
        ctx_pack = {}
        for b in range(B):
            ctx_pack[b] = persist.tile([HD, S], dt_in, name=f"ctxp_{b}", tag=f"ctxp_{b}")

        # ---- stage 1 chunk: projections (qT/kT/vT) + RoPE + v transpose ----
        def proj_chunk(c):
            c0 = TC * c
            xt = [None] * KE
            for k in range(KE):
                t = xt_pool.tile([128, TC], dt_in, name=f"xt_{c}_{k}", tag="xt")
                nc.sync.dma_start(t[:], xT.ap()[128 * k : 128 * (k + 1), c0 : c0 + TC])
                xt[k] = t
            cos_c = cs_pool.tile([HD, TC], F32, name="cos_c", tag="cos_c")
            nc.sync.dma_start(cos_c[:], cosT.ap()[:, c0 : c0 + TC])
            sin_c = cs_pool.tile([HD, TC], F32, name="sin_c", tag="sin_c")
            nc.sync.dma_start(sin_c[:], sinT.ap()[:, c0 : c0 + TC])

            psq = ps_a.tile([HD, TC], F32, name="psq", tag="ps_a")
            psk = ps_a.tile([HD, TC], F32, name="psk", tag="ps_a")
            psv = ps_b.tile([HD, TC], F32, name="psv", tag="ps_b")
            for ps, w_sb in ((psq, wq_sb), (psk, wk_sb), (psv, wv_sb)):
                for k in range(KE):
                    nc.tensor.matmul(
                        ps[:], w_sb[k][:], xt[k][:],
                        start=(k == 0), stop=(k == KE - 1),
                    )
            # q/k: bias evict on ACT, rotate matmul, rope combine DVE+GpSimd
            for nm, ps, b_sb, out in (
                ("q", psq, bq_sb, q_rope),
                ("k", psk, bk_sb, k_rope),
            ):
                raw = qkraw_pool.tile([HD, TC], dt_in, name=f"{nm}raw", tag="qkraw")
                nc.scalar.activation(raw[:], ps[:], AF.Identity, bias=b_sb[:, 0:1])
                psrot = ps_b.tile([HD, TC], F32, name="psrot", tag="ps_b")
                nc.tensor.matmul(psrot[:], rot_sb[:], raw[:], start=True, stop=True)
                sprod = rope_tmp.tile([HD, TC], F32, name="sprod", tag="ropetmp")
                nc.vector.tensor_tensor(sprod[:], psrot[:], sin_c[:], op=OP.mult)
                cprod = rope_tmp.tile([HD, TC], F32, name="cprod", tag="ropetmp")
                nc.vector.tensor_tensor(cprod[:], raw[:], cos_c[:], op=OP.mult)
                nc.gpsimd.tensor_tensor(
                    out[:, c0 : c0 + TC], cprod[:], sprod[:], op=OP.add
                )
            # vT -> v natural via PE transpose; bias evict on ACT
            vraw = qkraw_pool.tile([HD, TC], dt_in, name="vraw", tag="qkraw")
            nc.scalar.activation(vraw[:], psv[:], AF.Identity, bias=bv_sb[:, 0:1])
            for j in range(TC // 128):
                pvt = ps_b.tile([128, 128], dt_in, name="pvt", tag="ps_b")
                nc.tensor.transpose(pvt[:], vraw[:, 128 * j : 128 * (j + 1)], ident[:])
                vt = v_sb[(c0 + 128 * j) // 128]
                for h in range(HPC):
                    nc.vector.tensor_copy(
                        vt[:, (D + 1) * h : (D + 1) * h + D],
                        pvt[:, D * h : D * (h + 1)],
                    )

        # ---- stage 2 block: attention for one (batch, query-chunk) ----
        def qc_block(b, qc):
            t0 = b * S
            q0 = t0 + QC * qc
            psc = [
                ps_c.tile([D + 1, QC], F32, name=f"psctx{h}", tag="ps_b")
                for h in range(HPC)
            ]
            for kt in range(NKT):
                k0 = t0 + 128 * kt
                # both heads' scores into one 2-bank psum tile -> single exp
                pss = ps_a.tile([128, HPC * QC], F32, name="pss", tag="ps_a")
                for h in range(HPC):
                    nc.tensor.matmul(
                        pss[:, QC * h : QC * (h + 1)],
                        k_rope[D * h : D * (h + 1), k0 : k0 + 128],
                        q_rope[D * h : D * (h + 1), q0 : q0 + QC],
                        start=True, stop=True,
                    )
                ex = exps_pool.tile([128, HPC * QC], dt_in, name="ex", tag="exps")
                nc.scalar.activation(ex[:], pss[:], AF.Exp, scale=scale)
                for h in range(HPC):
                    nc.tensor.matmul(
                        psc[h][:],
                        v_sb[k0 // 128][:, (D + 1) * h : (D + 1) * (h + 1)],
                        ex[:, QC * h : QC * (h + 1)],
                        start=(kt == 0), stop=(kt == NKT - 1),
                    )
            # normalize: evict Z row + unnormalized ctx to SBUF (frees psc fast),
            # then 1/Z on DVE, broadcast via DRAM bounce, multiply
            for h in range(HPC):
                zrow = zr_pool.tile([1, QC], F32, name="zrow", tag="zrow")
                nc.vector.tensor_copy(zrow[:], psc[h][D : D + 1, :])
                cun = csh_pool.tile([D, QC], dt_in, name="cun", tag="csh")
                nc.vector.tensor_copy(cun[:], psc[h][0:D, :])
                zr = zr_pool.tile([1, QC], F32, name="zrec", tag="zr")
                nc.vector.reciprocal(zr[:], zrow[:])
                zd = dram.tile([QC], F32, name="zd", tag="zd")
                nc.gpsimd.dma_start(zd[:], zr[:])
                zb = zb_pool.tile([D, QC], F32, name="zb", tag="zb")
                zd_ap = zd[:]
                nc.gpsimd.dma_start(
                    zb[:],
                    bass.AP(
                        tensor=zd_ap.tensor, offset=zd_ap.offset,
                        ap=[[0, D], [1, QC]],
                    ),
                )
                if h == 0:
                    nc.vector.tensor_tensor(
                        ctx_pack[b][0:D, QC * qc : QC * (qc + 1)],
                        cun[:], zb[:], op=OP.mult,
                    )
                else:
                    csh = csh_pool.tile([D, QC], dt_in, name="csh2", tag="csh2")
                    nc.vector.tensor_tensor(csh[:], cun[:], zb[:], op=OP.mult)
                    nc.scalar.dma_start(
                        ctx_pack[b][D : 2 * D, QC * qc : QC * (qc + 1)], csh[:]
                    )

        # ---- stage 3: output projection for some query tiles of batch b ----
        def emit_out_proj(b, tiles):
            t0 = b * S
            for j in tiles:
                for e in range(E // 512):
                    pso = ps_b.tile([128, 512], F32, name="pso", tag="ps_b")
                    nc.tensor.matmul(
                        pso[:],
                        ctx_pack[b][:, 128 * j : 128 * (j + 1)],
                        wo_sb[:, 512 * e : 512 * (e + 1)],
                        start=True, stop=True,
                    )
                    osb = osb_pool.tile([128, 512], F32, name="osb", tag="osb")
                    nc.vector.tensor_copy(osb[:], pso[:])
                    eng = nc.sync if e == 0 else nc.gpsimd
                    eng.dma_start(
                        yp.ap()[t0 + 128 * j : t0 + 128 * (j + 1), 512 * e : 512 * (e + 1)],
                        osb[:],
                    )

        # ---- interleaved emission ----
        # batch-0 projections; then batch-1 projections interleaved with
        # batch-0 attention; then batch-1 attention interleaved with batch-0
        # output projection; then batch-1 output projection.
        JT = S // 128  # out-proj tiles per batch
        NCB = NCH // B  # projection chunks per batch
        if NCB >= 1 and NQC >= 1 and NCH == B * NCB:
            def oj(qc):
                return range(JT * qc // NQC, JT * (qc + 1) // NQC)

            for c in range(NCB):
                proj_chunk(c)
            # attention blocks in order; out-projection lags one block behind
            blocks = [(b, qc) for b in range(B) for qc in range(NQC)]
            for idx, (b, qc) in enumerate(blocks):
                if idx < NCB:
                    proj_chunk(NCB + idx)
                qc_block(b, qc)
                if idx >= 1:
                    pb, pqc = blocks[idx - 1]
                    emit_out_proj(pb, oj(pqc))
            lb, lqc = blocks[-1]
            emit_out_proj(lb, oj(lqc))
        else:
            for c in range(NCH):
                proj_chunk(c)
            for b in range(B):
                for qc in range(NQC):
                    qc_block(b, qc)
            for b in range(B):
                emit_out_proj(b, range(JT))

    nc.compile()
    return nc


def _rope_tables():
    inv_freq = 1.0 / (10000.0 ** (np.arange(0, D, 2, dtype=np.float32) / D))
    t = np.arange(S, dtype=np.float32)
    freqs = np.outer(t, inv_freq).astype(np.float32)
    emb = np.concatenate([freqs, freqs], axis=-1)
    return np.cos(emb).astype(np.float32), np.sin(emb).astype(np.float32)


def _rot_matrix():
    R = np.zeros((HD, HD), np.float32)
    for hh in range(HPC):
        for do in range(D):
            po = D * hh + do
            if do < D // 2:
                R[D * hh + do + D // 2, po] = -1.0
            else:
                R[D * hh + do - D // 2, po] = 1.0
    return R


def kernel(x, Wq, bq, Wk, bk, Wv, bv, Wo, bo):
    global LAST_RESULTS
    import ml_dtypes

    x = np.asarray(x, dtype=np.float32)
    Wq, bq = np.asarray(Wq, np.float32), np.asarray(bq, np.float32)
    Wk, bk = np.asarray(Wk, np.float32), np.asarray(bk, np.float32)
    Wv, bv = np.asarray(Wv, np.float32), np.asarray(bv, np.float32)
    Wo, bo = np.asarray(Wo, np.float32), np.asarray(bo, np.float32)

    mode = MM_MODE
    dt_np = ml_dtypes.bfloat16 if mode == "bf16" else np.float32
    T = B * S

    if mode not in _NC_CACHE:
        _NC_CACHE[mode] = build_mha_nc(mode)
    nc = _NC_CACHE[mode]

    xT = np.ascontiguousarray(x.reshape(T, E).T).astype(dt_np)
    cos, sin = _rope_tables()
    cosT = np.tile(np.ascontiguousarray(cos.T), (HPC, B)).astype(np.float32)
    sinT = np.tile(np.ascontiguousarray(sin.T), (HPC, B)).astype(np.float32)
    R = _rot_matrix().astype(dt_np)

    in_maps = []
    for c in range(N_CORES):
        sl = slice(HD * c, HD * (c + 1))
        in_maps.append(
            {
                "xT": xT,
                "wq": np.ascontiguousarray(Wq[:, sl]).astype(dt_np),
                "wk": np.ascontiguousarray(Wk[:, sl]).astype(dt_np),
                "wv": np.ascontiguousarray(Wv[:, sl]).astype(dt_np),
                "bq": np.ascontiguousarray(bq[sl][:, None]).astype(np.float32),
                "bk": np.ascontiguousarray(bk[sl][:, None]).astype(np.float32),
                "bv": np.ascontiguousarray(bv[sl][:, None]).astype(np.float32),
                "wo": np.ascontiguousarray(Wo[sl, :]).astype(dt_np),
                "cosT": cosT,
                "sinT": sinT,
                "rot": R,
                "ones": np.ones((1, 1), dt_np),
            }
        )

    res = bass_utils.run_bass_kernel_spmd(nc, in_maps, core_ids=list(range(N_CORES)))
    LAST_RESULTS = res

    out = np.zeros((T, E), np.float64)
    for c in range(N_CORES):
        out += res.results[c]["yp"].astype(np.float64)
    out += bo.astype(np.float64)
    return out.astype(np.float32).reshape(B, S, E)



# revision 29
# speedup vs baseline: 1.2994x; 1.2994x over previous
"""Trainium2 Bass kernel for nn_MultiHeadAttention (RoPE MHA, B=2 S=2048 E=1024 H=16).

Sharding: tensor-parallel over heads — 2 heads per core on 8 cores. Each core
computes its heads' q/k/v projections, RoPE, attention, and the partial output
projection (its rows of Wo); the host sums the 8 partials and adds bo.

Schedule (single pass, engine-order = emission order):
  stage1a: projection chunks 0-3 (batch 0) run back-to-back, PE-dense.
  blocks (0,0)..(0,3): attention on batch 0, ACT (exp) paced; chunks 4-7
    (batch 1 projections) are interleaved as PE fillers inside the kt loops.
  blocks (1,0)..(1,3): attention on batch 1 with batch-0 / early batch-1
    output-projection j-tiles interleaved as fillers.
  tail: last out-projection tiles.

PSUM budget (16KB/partition, exact): pss 2x4KB (scores ping-pong, also lends
slots to stage1a projections), psc 2x2KB (per-head ctx accumulators),
flex 2x2KB (interleaved projections / pvt transposes / out-proj tiles).

Softmax: exp on ACT with the row-sum obtained via a ones column appended to v
(no separate reduction). Normalization is deferred: unnormalized ctx is
evicted to SBUF, Z reciprocals (reciprocal_approx_fast, straight from the
PSUM row) are broadcast via a DRAM bounce, then one in-place multiply.
"""

import os
import sys
from contextlib import ExitStack

import numpy as np

for _p in ("/opt/trn_rl_repo", "/opt/pypackages"):
    if _p not in sys.path and os.path.isdir(_p):
        sys.path.append(_p)

import concourse.bass as bass
import concourse.mybir as mybir
import concourse.tile as tile
from concourse import bacc
from concourse import bass_utils
from concourse.masks import make_identity

F32 = mybir.dt.float32
BF16 = mybir.dt.bfloat16
AF = mybir.ActivationFunctionType
OP = mybir.AluOpType

B = 2
S = 2048
E = 1024
H = 16
D = 64
N_CORES = 8
HPC = H // N_CORES  # heads per core = 2
HD = HPC * D  # 128

T = B * S
TC = 512  # token chunk for projections
NCH = T // TC  # 8 chunks (4 per batch)
QC = 512  # query chunk in attention
NQC = S // QC  # 4 blocks per batch
NKT = S // 128  # 16 key tiles per batch
KE = E // 128  # 8 contraction tiles for projections
JT = S // 128  # 16 out-proj token tiles per batch

MM_MODE = "bf16"  # informational (test harness prints it)

LAST_RESULTS = None  # BassKernelResults of the most recent run (for test harness)
_NC_CACHE = {}


def build_mha_nc():
    nc = bacc.Bacc(None, target_bir_lowering=False, debug=False)

    xT = nc.dram_tensor("xT", [128, NCH, KE, TC], BF16, kind="ExternalInput")
    wq = nc.dram_tensor("wq", [E, HD], BF16, kind="ExternalInput")
    wk = nc.dram_tensor("wk", [E, HD], BF16, kind="ExternalInput")
    wv = nc.dram_tensor("wv", [E, HD], BF16, kind="ExternalInput")
    bq = nc.dram_tensor("bq", [HD, 1], F32, kind="ExternalInput")
    bk = nc.dram_tensor("bk", [HD, 1], F32, kind="ExternalInput")
    bv = nc.dram_tensor("bv", [HD, 1], F32, kind="ExternalInput")
    wo = nc.dram_tensor("wo", [HD, E], BF16, kind="ExternalInput")
    cosS = nc.dram_tensor("cosS", [128, S], F32, kind="ExternalInput")
    sinS = nc.dram_tensor("sinS", [128, S], F32, kind="ExternalInput")
    rot = nc.dram_tensor("rot", [HD, HD], BF16, kind="ExternalInput")
    yp = nc.dram_tensor("yp", [T, E], BF16, kind="ExternalOutput")

    scale = 1.0 / np.sqrt(D)

    with tile.TileContext(nc) as tc, ExitStack() as ctx:
        const = ctx.enter_context(tc.tile_pool(name="const", bufs=1))
        xt_pool = ctx.enter_context(tc.tile_pool(name="xt", bufs=16))
        raw_pool = ctx.enter_context(tc.tile_pool(name="raw", bufs=4))
        prod_pool = ctx.enter_context(tc.tile_pool(name="prod", bufs=8))
        persist = ctx.enter_context(tc.tile_pool(name="persist", bufs=1))
        exps_pool = ctx.enter_context(tc.tile_pool(name="exps", bufs=8))
        z_pool = ctx.enter_context(tc.tile_pool(name="z", bufs=4))
        osb_pool = ctx.enter_context(tc.tile_pool(name="osb", bufs=6))
        dram = ctx.enter_context(tc.tile_pool(name="dram", bufs=4, space="DRAM"))
        ps = ctx.enter_context(tc.tile_pool(name="ps", bufs=1, space="PSUM"))

        # ---- x tiles: one strided DMA per chunk [128, KE, TC] ----
        xt_tiles = {}

        def load_xt(c):
            if c in xt_tiles or c >= NCH:
                return
            t = xt_pool.tile([128, KE, TC], BF16, name=f"xt_{c}", tag="xt", bufs=3)
            h = KE // 2
            nc.sync.dma_start(t[:, 0:h, :], xT.ap()[:, c, 0:h, :])
            nc.scalar.dma_start(t[:, h:KE, :], xT.ap()[:, c, h:KE, :])
            xt_tiles[c] = t

        # chunk 0 split three ways so the PE can start projections sooner;
        # its gpsimd slice is issued right after make_identity below.
        xt0 = xt_pool.tile([128, KE, TC], BF16, name="xt_0", tag="xt", bufs=3)
        nc.sync.dma_start(xt0[:, 0:3, :], xT.ap()[:, 0, 0:3, :])
        nc.scalar.dma_start(xt0[:, 3:6, :], xT.ap()[:, 0, 3:6, :])
        xt_tiles[0] = xt0
        load_xt(1)
        # ---- ident + PE warmup first (ramps the PE pstate while DMAs land)
        ident = const.tile([128, 128], BF16, name="ident", tag="ident")
        make_identity(nc, ident)
        nc.gpsimd.dma_start(xt0[:, 6:8, :], xT.ap()[:, 0, 6:8, :])
        warm = ps.tile([128, 128], F32, name="warm", tag="flex", bufs=2)
        for _ in range(40):
            nc.tensor.matmul(warm[:], ident[:], ident[:], start=True, stop=True)
        # ---- weights: one strided DMA each (wq/wk on gpsimd, wv on sync) ----
        w_all = {}
        for nm, dr, eng in (
            ("wq", wq, nc.gpsimd),
            ("wk", wk, nc.gpsimd),
            ("wv", wv, nc.sync),
        ):
            t = const.tile([128, KE, HD], BF16, name=f"{nm}_all", tag=f"{nm}_all")
            dap = dr.ap()
            eng.dma_start(
                t[:],
                bass.AP(
                    tensor=dap.tensor,
                    offset=dap.offset,
                    ap=[[HD, 128], [128 * HD, KE], [1, HD]],
                ),
            )
            w_all[nm] = t
        wq_sb = [w_all["wq"][:, k, :] for k in range(KE)]
        wk_sb = [w_all["wk"][:, k, :] for k in range(KE)]
        wv_sb = [w_all["wv"][:, k, :] for k in range(KE)]
        bq_sb = const.tile([HD, 1], F32, name="bq_sb", tag="bq_sb")
        nc.scalar.dma_start(bq_sb[:], bq.ap())
        bk_sb = const.tile([HD, 1], F32, name="bk_sb", tag="bk_sb")
        nc.scalar.dma_start(bk_sb[:], bk.ap())
        ones_col = const.tile([1, 128], BF16, name="ones_col", tag="ones_col")
        nc.vector.memset(ones_col[:], 1.0)

        # cos/sin [128, S], pre-replicated host-side; scalar queue (ACT
        # is idle during stage1a and these are needed within ~4us).
        cos_sb = const.tile([128, S], F32, name="cos_sb", tag="cos_sb")
        nc.scalar.dma_start(cos_sb[:], cosS.ap())
        bv_sb = const.tile([HD, 1], F32, name="bv_sb", tag="bv_sb")
        nc.scalar.dma_start(bv_sb[:], bv.ap())
        rot_sb = const.tile([HD, HD], BF16, name="rot_sb", tag="rot_sb")
        nc.scalar.dma_start(rot_sb[:], rot.ap())
        sin_sb = const.tile([128, S], F32, name="sin_sb", tag="sin_sb")
        nc.scalar.dma_start(sin_sb[:], sinS.ap())
        wo_sb = const.tile([HD, E], BF16, name="wo_sb", tag="wo_sb")
        nc.scalar.dma_start(wo_sb[:], wo.ap())

        # ---- persistent intermediates ----
        q_rope = persist.tile([HD, T], BF16, name="q_rope", tag="q_rope")
        k_rope = persist.tile([HD, T], BF16, name="k_rope", tag="k_rope")
        v_all = persist.tile([128, T // 128, HPC * (D + 1)], BF16, name="v_all", tag="v_all")
        for h in range(HPC):
            nc.vector.memset(v_all[:, :, (D + 1) * h + D : (D + 1) * (h + 1)], 1.0)
        ctx_pack = {}
        for b in range(B):
            ctx_pack[b] = persist.tile([HD, S], BF16, name=f"ctxp_{b}", tag=f"ctxp_{b}")

        # ---- stage 1: projection chunk phases ----
        # Each chunk = 3 phases: (xt load + q-proj + rope), (k-proj + rope),
        # (v-proj + transpose). use_pss=True borrows pss-tag slots (stage1a,
        # attention not running); otherwise flex-tag slots (interleaved).

        rope_defer = []

        def drain_prev_rope():
            # run rope combines deferred from earlier phases (keeps the DVE
            # queue clear of cos/sin-dependent work ahead of PE-feeding evicts)
            while len(rope_defer) > 1:
                rope_defer.pop(0)()
            if rope_defer:
                rope_defer.pop(0)()

        pj_psum = {}

        def phase_q(c, use_pss, which, k0_=0, k1_=KE):
            c0 = TC * c
            s0 = c0 % S  # position within batch for rope tables
            if which == "q" and k0_ == 0:
                load_xt(c)
                load_xt(c + 1)
            xt = xt_tiles[c]
            w_sb, b_sb, out = (
                (wq_sb, bq_sb, q_rope) if which == "q" else (wk_sb, bk_sb, k_rope)
            )
            key = (c, which)
            if key in pj_psum:
                psq, psrot = pj_psum[key]
            else:
                if use_pss and which == "q":
                    big = ps.tile([128, 2 * TC], F32, name=f"pj_{which}{c}", tag="pss", bufs=2)
                    psq = big[:, 0:TC]
                    psrot = big[:, TC : 2 * TC]
                elif use_pss:  # k-phase: borrow the psc slots
                    f1 = ps.tile([128, TC], F32, name=f"pjq_{which}{c}", tag="psc", bufs=2)
                    f2 = ps.tile([128, TC], F32, name=f"pjr_{which}{c}", tag="psc", bufs=2)
                    psq = f1[:]
                    psrot = f2[:]
                else:
                    f1 = ps.tile([128, TC], F32, name=f"pjq_{which}{c}", tag="flex", bufs=2)
                    f2 = ps.tile([128, TC], F32, name=f"pjr_{which}{c}", tag="flex", bufs=2)
                    psq = f1[:]
                    psrot = f2[:]
                pj_psum[key] = (psq, psrot)
            if k1_ >= KE:
                del pj_psum[key]
            for k in range(k0_, k1_):
                nc.tensor.matmul(
                    psq, w_sb[k], xt[:, k, :],
                    start=(k == 0), stop=(k == KE - 1),
                )
            if k1_ < KE:
                return
            raw = raw_pool.tile([HD, TC], BF16, name=f"{which}raw", tag="raw")
            nc.vector.tensor_scalar_add(raw[:], psq, b_sb[:, 0:1])
            nc.tensor.matmul(psrot, rot_sb[:], raw[:], start=True, stop=True)
            drain_prev_rope()

            def rope_combine(raw=raw, psrot=psrot, out=out, c0=c0, s0=s0):
                cprod = prod_pool.tile([HD, TC], BF16, name="cprod", tag="prod")
                nc.vector.tensor_tensor(
                    cprod[:], raw[:], cos_sb[:, s0 : s0 + TC], op=OP.mult
                )
                sprod = prod_pool.tile([HD, TC], BF16, name="sprod", tag="prod")
                nc.vector.tensor_tensor(
                    sprod[:], psrot, sin_sb[:, s0 : s0 + TC], op=OP.mult
                )
                nc.gpsimd.tensor_tensor(
                    out[:, c0 : c0 + TC], cprod[:], sprod[:], op=OP.add
                )

            rope_defer.append(rope_combine)

        def phase_v(c, use_pss, k0_=0, k1_=KE):
            c0 = TC * c
            xt = xt_tiles[c]
            key = (c, "v")
            if key in pj_psum:
                psv = pj_psum[key][0]
            else:
                f1 = ps.tile([128, TC], F32, name=f"pjv_{c}", tag="flex", bufs=2)
                psv = f1[:]
                pj_psum[key] = (psv, None)
            for k in range(k0_, k1_):
                nc.tensor.matmul(
                    psv, wv_sb[k], xt[:, k, :], start=(k == 0), stop=(k == KE - 1)
                )
            if k1_ < KE:
                return
            del pj_psum[key]
            vraw = raw_pool.tile([HD, TC], BF16, name="vraw", tag="raw")
            nc.vector.tensor_scalar_add(vraw[:], psv, bv_sb[:, 0:1])
            pvt = ps.tile([128, TC // 128, 128], BF16, name=f"pvt_{c}", tag="flex", bufs=2)
            for j in range(TC // 128):
                nc.tensor.transpose(
                    pvt[:, j, :],
                    vraw[:, 128 * j : 128 * (j + 1)],
                    ident[:],
                )
            i0 = c0 // 128
            for h in range(HPC):
                nc.vector.tensor_copy(
                    v_all[:, i0 : i0 + TC // 128, (D + 1) * h : (D + 1) * h + D],
                    pvt[:, :, D * h : D * (h + 1)],
                )
            del xt_tiles[c]
            drain_prev_rope()

        def chunk_fillers(c, use_pss=False):
            u = use_pss
            return [
                lambda: phase_q(c, u, "q", 0, 4),
                lambda: phase_q(c, u, "q", 4, KE),
                lambda: phase_q(c, u, "k", 0, 4),
                lambda: phase_q(c, u, "k", 4, KE),
                lambda: phase_v(c, u, 0, 4),
                lambda: phase_v(c, u, 4, KE),
            ]

        # ---- out-projection piece: one token j-tile of batch b ----
        def op_piece(b, j, tail=False):
            t0 = b * S
            for e in range(E // 512):
                ptag = "pss" if (tail and e == 1) else "flex"
                pso = ps.tile([128, 512], F32, name=f"pso_{b}_{j}_{e}", tag=ptag, bufs=2)
                nc.tensor.matmul(
                    pso[:],
                    ctx_pack[b][:, 128 * j : 128 * (j + 1)],
                    wo_sb[:, 512 * e : 512 * (e + 1)],
                    start=True, stop=True,
                )
                osb = osb_pool.tile([128, 512], BF16, name="osb", tag="osb")
                if tail and e == 0:  # ACT is idle after the last exp
                    nc.scalar.activation(osb[:], pso[:], AF.Identity, bias=0.0)
                else:
                    nc.vector.tensor_copy(osb[:], pso[:])
                if tail:
                    eng = (nc.sync, nc.scalar)[(2 * j + e) % 2]
                else:
                    eng = nc.sync if e == 0 else nc.gpsimd
                eng.dma_start(
                    yp.ap()[t0 + 128 * j : t0 + 128 * (j + 1), 512 * e : 512 * (e + 1)],
                    osb[:],
                )

        # ---- stage 2: attention block for (batch, query-chunk) ----
        def qc_block(b, qc, fillers):
            t0 = b * S
            q0 = t0 + QC * qc
            psc = [
                ps.tile([D + 1, QC], F32, name=f"psc{h}_{b}_{qc}", tag="psc", bufs=2)
                for h in range(HPC)
            ]
            nf = len(fillers)
            fi = 0
            for kt in range(NKT):
                k0 = t0 + 128 * kt
                pss = ps.tile([128, HPC * QC], F32, name=f"pss_{kt}", tag="pss", bufs=2)
                for h in range(HPC):
                    nc.tensor.matmul(
                        pss[:, QC * h : QC * (h + 1)],
                        k_rope[D * h : D * (h + 1), k0 : k0 + 128],
                        q_rope[D * h : D * (h + 1), q0 : q0 + QC],
                        start=True, stop=True,
                    )
                ex = exps_pool.tile([128, HPC * QC], BF16, name="ex", tag="exps")
                nc.scalar.activation(ex[:], pss[:], AF.Exp, scale=scale)
                for h in range(HPC):
                    nc.tensor.matmul(
                        psc[h][:],
                        v_all[:, k0 // 128, (D + 1) * h : (D + 1) * (h + 1)],
                        ex[:, QC * h : QC * (h + 1)],
                        start=(kt == 0), stop=(kt == NKT - 1),
                    )
                while fi * NKT < nf * (kt + 1):
                    fillers[fi]()
                    fi += 1
            assert fi == nf
            # Z reciprocals straight from the PSUM rows, then evict ctx
            # (h0 on ACT, h1 on DVE so both psc slots free fast).
            zrow = z_pool.tile([1, HPC * QC], F32, name="zrow", tag="zr")
            for h in range(HPC):
                nc.vector.tensor_copy(
                    zrow[0:1, QC * h : QC * (h + 1)], psc[h][D : D + 1, :]
                )
            zpack = z_pool.tile([1, HPC * QC], F32, name="zpack", tag="zp")
            nc.vector.reciprocal_approx_fast(zpack[:], zrow[:])
            cs = slice(QC * qc, QC * (qc + 1))
            nc.scalar.copy(ctx_pack[b][0:D, cs], psc[0][0:D, :])
            nc.vector.tensor_copy(ctx_pack[b][D : 2 * D, cs], psc[1][0:D, :])
            # broadcast 1/Z over the 64 partitions of each head via DRAM bounce
            zd = dram.tile([HPC * QC], F32, name="zd", tag="zd")
            nc.gpsimd.dma_start(zd[:], zpack[:])
            zb2 = z_pool.tile([128, QC], F32, name="zb2", tag="zb")
            zd_ap = zd[:]
            nc.gpsimd.dma_start(
                zb2[:],
                bass.AP(
                    tensor=zd_ap.tensor,
                    offset=zd_ap.offset,
                    ap=[[QC, HPC], [0, D], [1, QC]],
                ),
            )
            nc.vector.tensor_tensor(
                ctx_pack[b][:, cs], ctx_pack[b][:, cs], zb2[:], op=OP.mult
            )

        # ---- emission ----
        for c in range(4):  # batch-0 projections, PE-dense head
            for f in chunk_fillers(c, use_pss=True):
                f()
        # batch-0 attention with batch-1 projections as fillers
        for qc in range(NQC):
            qc_block(0, qc, chunk_fillers(4 + qc, use_pss=False))
        # batch-1 attention with out-projection fillers
        qc_block(1, 0, [lambda j=j: op_piece(0, j) for j in range(0, 8)])
        qc_block(1, 1, [lambda j=j: op_piece(0, j) for j in range(8, 16)])
        qc_block(1, 2, [lambda j=j: op_piece(1, j) for j in range(0, 8)])
        qc_block(1, 3, [lambda j=j: op_piece(1, j) for j in range(8, 12)])
        for j in range(12, 16):
            op_piece(1, j, tail=True)

    nc.compile()
    return nc


def _rope_tables():
    inv_freq = 1.0 / (10000.0 ** (np.arange(0, D, 2, dtype=np.float32) / D))
    t = np.arange(S, dtype=np.float32)
    freqs = np.outer(t, inv_freq).astype(np.float32)  # [S, 32]
    return np.cos(freqs).astype(np.float32), np.sin(freqs).astype(np.float32)


def _rot_matrix():
    R = np.zeros((HD, HD), np.float32)
    for hh in range(HPC):
        for do in range(D):
            po = D * hh + do
            if do < D // 2:
                R[D * hh + do + D // 2, po] = -1.0
            else:
                R[D * hh + do - D // 2, po] = 1.0
    return R


def kernel(x, Wq, bq, Wk, bk, Wv, bv, Wo, bo):
    global LAST_RESULTS
    import ml_dtypes

    x = np.asarray(x, dtype=np.float32)
    Wq, bq = np.asarray(Wq, np.float32), np.asarray(bq, np.float32)
    Wk, bk = np.asarray(Wk, np.float32), np.asarray(bk, np.float32)
    Wv, bv = np.asarray(Wv, np.float32), np.asarray(bv, np.float32)
    Wo, bo = np.asarray(Wo, np.float32), np.asarray(bo, np.float32)

    dt_np = ml_dtypes.bfloat16

    if "nc" not in _NC_CACHE:
        _NC_CACHE["nc"] = build_mha_nc()
    nc = _NC_CACHE["nc"]

    xTf = x.reshape(T, E).T.astype(dt_np)  # [E, T]
    # chunk-major tiling: [p, c, k, j] = xT[128k+p, 512c+j]
    xT = np.ascontiguousarray(
        xTf.reshape(KE, 128, NCH, TC).transpose(1, 2, 0, 3)
    )
    cos, sin = _rope_tables()  # [S, 32]
    cosS = np.ascontiguousarray(np.tile(cos.T, (4, 1)))  # [128, S]
    sinS = np.ascontiguousarray(np.tile(sin.T, (4, 1)))
    R = _rot_matrix().astype(dt_np)

    in_maps = []
    for c in range(N_CORES):
        sl = slice(HD * c, HD * (c + 1))
        in_maps.append(
            {
                "xT": xT,
                "wq": np.ascontiguousarray(Wq[:, sl]).astype(dt_np),
                "wk": np.ascontiguousarray(Wk[:, sl]).astype(dt_np),
                "wv": np.ascontiguousarray(Wv[:, sl]).astype(dt_np),
                "bq": np.ascontiguousarray(bq[sl][:, None]).astype(np.float32),
                "bk": np.ascontiguousarray(bk[sl][:, None]).astype(np.float32),
                "bv": np.ascontiguousarray(bv[sl][:, None]).astype(np.float32),
                "wo": np.ascontiguousarray(Wo[sl, :]).astype(dt_np),
                "cosS": cosS,
                "sinS": sinS,
                "rot": R,
            }
        )

    res = bass_utils.run_bass_kernel_spmd(nc, in_maps, core_ids=list(range(N_CORES)))
    LAST_RESULTS = res

    out = np.zeros((T, E), np.float64)
    for c in range(N_CORES):
        out += res.results[c]["yp"].astype(np.float64)
    out += bo.astype(np.float64)
    return out.astype(np.float32).reshape(B, S, E)
